# Initial kernel scaffold
#
"""Trainium2 Bass kernel for nn_Aux2_46969762349381 (scatter_memory).

Computes, for embs [32, 2048, 1024] f32:
  status_probs = softmax(embs @ W_status.T + b_status)   # [B,T,5]
  flight_probs = softmax(embs @ W_flight.T + b_flight)   # [B,T,30]
  out = concat([s0, s2, s1, s4*flight, s3*flight], -1)   # [B,T,63]

Strategy (pure data parallel over batch, 8 cores; full inputs in, full
output out):
  - each core owns 4 batches = 8192 tokens, token t = p*64 + i
    (p = SBUF partition, i = token-tile index) so both the embs loads and
    the out stores are contiguous >=4KB per partition.
  - embs tiles load naturally [128 tok, 1024 emb]; PE transpose (identity
    matmul) flips 128x128 blocks into PSUM; DVE/ACT copy them to SBUF
    giving embsT [128 emb, 8*512 tok].
  - 8 accumulating matmuls (lhsT = host-pretransposed W [128,35] per
    emb-chunk, rhs = embsT chunk [128, 512], float32r) -> psum [35, 512]
    logits.T per 512-token group.
  - ScalarE exp reads the PSUM logits with the per-partition class bias
    fused into the activation -> expT [35, 512] in SBUF.
  - PE transposes expT back to [128 tok, 35] PSUM; DVE does the softmax
    normalization + outer-product scatter into [128, ntile, 63] and the
    result DMAs out via SWDGE.
"""

import os
import sys

import numpy as np

for _p in ("/opt/trn_rl_repo", "/root/.axon_site/_ro/trn_rl_repo"):
    if os.path.isdir(_p) and _p not in sys.path:
        sys.path.insert(0, _p)

from contextlib import ExitStack

import concourse.bass as bass
import concourse.tile as tile
from concourse import mybir
from concourse.bass_utils import run_bass_kernel_spmd

N_CORES = 8
B, T, E = 32, 2048, 1024
NS, NF = 5, 30
NCLS = NS + NF          # 35 combined classes
OUTC = 63
P = 128                 # SBUF partitions
ECH = E // P            # 8 emb chunks of 128
GT = 4                  # token tiles (of 128 tokens) per matmul group
GTOK = GT * P           # 512 tokens per group
AG = 2                  # groups per assembly batch
F32 = mybir.dt.float32
F32R = mybir.dt.float32r
EXP = mybir.ActivationFunctionType.Exp


def _split_multiwait(nc, max_waits=1):
    """Workaround for this walrus build rejecting >1 sem-wait on one
    instruction: move extra waits onto single-wait NoOps just before it."""
    for bb in nc.m.functions[0].blocks:
        insts = list(bb.instructions)
        new_list = []
        changed = False
        for inst in insts:
            si = inst.sync_info
            if si is not None and si.on_wait and len(si.on_wait) > max_waits:
                waits = list(si.on_wait)
                for w in waits[:-max_waits]:
                    nop = mybir.InstNoOp(
                        name=nc.get_next_instruction_name(),
                        ins=[],
                        outs=[],
                        engine=inst.engine,
                        sync_info=mybir.SyncInfo(on_wait=[w], on_update=[]),
                    )
                    nc.register_instruction(nop)
                    new_list.append(nop)
                    changed = True
                inst.sync_info = mybir.SyncInfo(
                    on_wait=waits[-max_waits:], on_update=list(si.on_update)
                )
            new_list.append(inst)
        if changed:
            bb.instructions = new_list


def build_program(tok, copy_split=5, mm_dtype=F32R, tr_dtype=F32):
    """Build the per-core Bass program for `tok` tokens (tok % 1024 == 0)."""
    S = tok // P            # token tiles per core
    n_groups = S // GT
    n_batches = n_groups // AG
    NT = AG * GT            # tiles per assembly batch (8)

    nc = bass.Bass("TRN2", num_devices=N_CORES)
    embs_d = nc.dram_tensor("embs", [tok, E], F32, kind="ExternalInput")
    w_d = nc.dram_tensor("wt", [P, ECH * NCLS], F32, kind="ExternalInput")
    b_d = nc.dram_tensor("bias", [NCLS, 1], F32, kind="ExternalInput")
    id_d = nc.dram_tensor("ident", [P, P], F32, kind="ExternalInput")
    out_d = nc.dram_tensor("out", [tok, OUTC], F32, kind="ExternalOutput")

    with tile.TileContext(nc) as tc, ExitStack() as ctx:
        consts = ctx.enter_context(tc.tile_pool(name="consts", bufs=1))
        emb_pool = ctx.enter_context(tc.tile_pool(name="emb", bufs=8))
        embT_pool = ctx.enter_context(tc.tile_pool(name="embT", bufs=2))
        expT_pool = ctx.enter_context(tc.tile_pool(name="expT", bufs=2))
        small = ctx.enter_context(tc.tile_pool(name="small", bufs=2))
        outsb = ctx.enter_context(tc.tile_pool(name="outsb", bufs=2))
        psT_pool = ctx.enter_context(tc.tile_pool(name="psT", bufs=4, space="PSUM"))
        psmm_pool = ctx.enter_context(tc.tile_pool(name="psmm", bufs=2, space="PSUM"))
        psxb_pool = ctx.enter_context(tc.tile_pool(name="psxb", bufs=2, space="PSUM"))

        w_sb = consts.tile([P, ECH * NCLS], F32)
        nc.sync.dma_start(w_sb[:], w_d.ap())
        b_sb = consts.tile([NCLS, 1], F32)
        nc.sync.dma_start(b_sb[:], b_d.ap())
        id_sb = consts.tile([P, P], F32)
        nc.sync.dma_start(id_sb[:], id_d.ap())

        embs_v = embs_d.ap().rearrange("(p i) e -> p i e", p=P, i=S)
        out_v = out_d.ap().rearrange("(p i) c -> p i c", p=P, i=S)

        for ab in range(n_batches):
            ps_xb = psxb_pool.tile([P, NT * NCLS], F32)
            for g2 in range(AG):
                g = ab * AG + g2
                embT = embT_pool.tile([P, ECH * GTOK], F32)
                embT_v = embT[:].rearrange("p (j t) -> p j t", t=GTOK)
                copy_k = 0
                for c in range(GT):
                    i = g * GT + c
                    emb_nat = emb_pool.tile([P, E], F32)
                    nc.sync.dma_start(emb_nat[:], embs_v[:, i, :])
                    for half in range(2):
                        psT = psT_pool.tile([P, 4 * P], F32)
                        for jj in range(4):
                            j = half * 4 + jj
                            src = emb_nat[:, j * P:(j + 1) * P]
                            if tr_dtype is not F32:
                                src = src.bitcast(tr_dtype)
                            nc.tensor.matmul(
                                psT[:, jj * P:(jj + 1) * P].bitcast(tr_dtype)
                                if tr_dtype is not F32
                                else psT[:, jj * P:(jj + 1) * P],
                                src,
                                id_sb[:].bitcast(tr_dtype)
                                if tr_dtype is not F32
                                else id_sb[:],
                                is_transpose=True,
                            )
                        dst = embT_v[:, half * 4:(half + 1) * 4, c * P:(c + 1) * P]
                        src = psT[:].rearrange("p (j t) -> p j t", j=4)
                        if copy_k < copy_split:
                            nc.scalar.copy(dst, src)
                        else:
                            nc.vector.tensor_copy(dst, src)
                        copy_k += 1
                ps_mm = psmm_pool.tile([NCLS, GTOK], F32)
                for j in range(ECH):
                    nc.tensor.matmul(
                        ps_mm[:],
                        w_sb[:, j * NCLS:(j + 1) * NCLS].bitcast(mm_dtype)
                        if mm_dtype is not F32
                        else w_sb[:, j * NCLS:(j + 1) * NCLS],
                        embT[:, j * GTOK:(j + 1) * GTOK].bitcast(mm_dtype)
                        if mm_dtype is not F32
                        else embT[:, j * GTOK:(j + 1) * GTOK],
                        start=(j == 0),
                        stop=(j == ECH - 1),
                    )
                expT = expT_pool.tile([NCLS, GTOK], F32)
                nc.scalar.activation(expT[:], ps_mm[:], EXP, bias=b_sb[:], scale=1.0)
                for c in range(GT):
                    it = g2 * GT + c
                    nc.tensor.matmul(
                        ps_xb[:, it * NCLS:(it + 1) * NCLS],
                        expT[:, c * P:(c + 1) * P],
                        id_sb[0:NCLS, 0:NCLS],
                        is_transpose=True,
                    )

            # ---- assembly for NT tiles (tokens p*64 + ab*NT .. +NT) ----
            X = ps_xb[:].rearrange("p (i c) -> p i c", c=NCLS)  # [128, NT, 35]
            sums = small.tile([P, 2 * NT], F32)
            nc.vector.reduce_sum(sums[:, 0:NT], X[:, :, 0:NS], axis=mybir.AxisListType.X)
            nc.vector.reduce_sum(
                sums[:, NT:2 * NT], X[:, :, NS:NCLS], axis=mybir.AxisListType.X
            )
            inv = small.tile([P, 2 * NT], F32)
            nc.vector.reciprocal(inv[:], sums[:])
            inv_s = inv[:, 0:NT]
            inv_f = inv[:, NT:2 * NT]
            dd = small.tile([P, NT], F32)
            nc.vector.tensor_mul(dd[:], inv_s, inv_f)
            cc = small.tile([P, 2 * NT], F32)
            nc.vector.tensor_mul(cc[:, 0:NT], X[:, :, 4], dd[:])  # book scale
            nc.vector.tensor_mul(cc[:, NT:2 * NT], X[:, :, 3], dd[:])  # change
            o_sb = outsb.tile([P, NT * OUTC], F32)
            O = o_sb[:].rearrange("p (i c) -> p i c", c=OUTC)
            inv_s3 = inv_s.unsqueeze(2)
            nc.vector.tensor_mul(O[:, :, 0:1], X[:, :, 0:1], inv_s3)
            nc.vector.tensor_mul(O[:, :, 1:2], X[:, :, 2:3], inv_s3)
            nc.vector.tensor_mul(O[:, :, 2:3], X[:, :, 1:2], inv_s3)
            nc.vector.tensor_mul(
                O[:, :, 3:3 + NF],
                X[:, :, NS:NCLS],
                cc[:, 0:NT].unsqueeze(2).broadcast_to((P, NT, NF)),
            )
            nc.vector.tensor_mul(
                O[:, :, 3 + NF:OUTC],
                X[:, :, NS:NCLS],
                cc[:, NT:2 * NT].unsqueeze(2).broadcast_to((P, NT, NF)),
            )
            nc.gpsimd.dma_start(out_v[:, ab * NT:(ab + 1) * NT, :], o_sb[:].rearrange("p (i c) -> p i c", c=OUTC))

    _split_multiwait(nc)
    return nc


def host_inputs(W_status, b_status, W_flight, b_flight):
    W = np.concatenate([np.asarray(W_status), np.asarray(W_flight)], axis=0)
    W = np.ascontiguousarray(W, dtype=np.float32)          # [35, 1024]
    # w_host[p, j*35 + c] = W[c, j*128 + p]
    w_host = np.ascontiguousarray(
        W.T.reshape(ECH, P, NCLS).transpose(1, 0, 2).reshape(P, ECH * NCLS)
    )
    b_host = np.ascontiguousarray(
        np.concatenate([np.asarray(b_status), np.asarray(b_flight)]).reshape(NCLS, 1),
        dtype=np.float32,
    )
    ident = np.eye(P, dtype=np.float32)
    return w_host, b_host, ident


def kernel(embs, W_status, b_status, W_flight, b_flight, **run_kwargs):
    embs = np.ascontiguousarray(np.asarray(embs), dtype=np.float32)
    tok = embs.shape[0] * embs.shape[1] // N_CORES
    w_host, b_host, ident = host_inputs(W_status, b_status, W_flight, b_flight)

    nc = build_program(tok)

    embs_flat = embs.reshape(-1, E)
    in_maps = [
        {
            "embs": embs_flat[c * tok:(c + 1) * tok],
            "wt": w_host,
            "bias": b_host,
            "ident": ident,
        }
        for c in range(N_CORES)
    ]
    res = run_bass_kernel_spmd(
        nc, in_maps, core_ids=list(range(N_CORES)), **run_kwargs
    )
    out = np.concatenate([res.results[c]["out"] for c in range(N_CORES)], axis=0)
    out = out.reshape(embs.shape[0], embs.shape[1], OUTC)
    if run_kwargs:
        return out, res
    return out


# revision 8
# speedup vs baseline: 1.1616x; 1.1616x over previous
"""Trainium2 Bass kernel for nn_Aux2_46969762349381 (scatter_memory).

Computes, for embs [32, 2048, 1024] f32:
  status_probs = softmax(embs @ W_status.T + b_status)   # [B,T,5]
  flight_probs = softmax(embs @ W_flight.T + b_flight)   # [B,T,30]
  out = concat([s0, s2, s1, s4*flight, s3*flight], -1)   # [B,T,63]

Strategy (pure data parallel over batch, 8 cores; full inputs in, full
output out):
  - each core owns 4 batches = 8192 tokens, token t = p*64 + i
    (p = SBUF partition, i = token-tile index) so both the embs loads and
    the out stores are contiguous >=4KB per partition.
  - embs tiles load naturally [128 tok, 1024 emb]; PE transpose (identity
    matmul) flips 128x128 blocks into PSUM; DVE/ACT copy them to SBUF
    giving embsT [128 emb, 8*512 tok].
  - 8 accumulating matmuls (lhsT = host-pretransposed W [128,35] per
    emb-chunk, rhs = embsT chunk [128, 512], float32r) -> psum [35, 512]
    logits.T per 512-token group.
  - ScalarE exp reads the PSUM logits with the per-partition class bias
    fused into the activation -> expT [35, 512] in SBUF.
  - PE transposes expT back to [128 tok, 35] PSUM; DVE does the softmax
    normalization + outer-product scatter into [128, ntile, 63] and the
    result DMAs out via SWDGE.
"""

import os
import sys

import numpy as np

for _p in ("/opt/trn_rl_repo", "/root/.axon_site/_ro/trn_rl_repo"):
    if os.path.isdir(_p) and _p not in sys.path:
        sys.path.insert(0, _p)

from contextlib import ExitStack

import concourse.bass as bass
import concourse.tile as tile
from concourse import mybir
from concourse.bass_utils import run_bass_kernel_spmd

N_CORES = 8
B, T, E = 32, 2048, 1024
NS, NF = 5, 30
NCLS = NS + NF          # 35 combined classes
OUTC = 63
P = 128                 # SBUF partitions
ECH = E // P            # 8 emb chunks of 128
GT = 4                  # token tiles (of 128 tokens) per matmul group
GTOK = GT * P           # 512 tokens per group
AG = 2                  # groups per assembly batch
F32 = mybir.dt.float32
F32R = mybir.dt.float32r
EXP = mybir.ActivationFunctionType.Exp


def _split_multiwait(nc, max_waits=1):
    """Workaround for this walrus build rejecting >1 sem-wait on one
    instruction: move extra waits onto single-wait NoOps just before it."""
    for bb in nc.m.functions[0].blocks:
        insts = list(bb.instructions)
        new_list = []
        changed = False
        for inst in insts:
            si = inst.sync_info
            if si is not None and si.on_wait and len(si.on_wait) > max_waits:
                waits = list(si.on_wait)
                for w in waits[:-max_waits]:
                    nop = mybir.InstNoOp(
                        name=nc.get_next_instruction_name(),
                        ins=[],
                        outs=[],
                        engine=inst.engine,
                        sync_info=mybir.SyncInfo(on_wait=[w], on_update=[]),
                    )
                    nc.register_instruction(nop)
                    new_list.append(nop)
                    changed = True
                inst.sync_info = mybir.SyncInfo(
                    on_wait=waits[-max_waits:], on_update=list(si.on_update)
                )
            new_list.append(inst)
        if changed:
            bb.instructions = new_list


def build_program(tok, copy_split=5, mm_dtype=F32R, tr_dtype=F32, loop_reps=0,
                  passes=1):
    """Build the per-core Bass program for `tok` tokens (tok % 1024 == 0).

    loop_reps > 0 wraps the whole body in a hardware For_i loop executing it
    that many times — benchmarking only (the axon dispatch overhead is ~80ms,
    so single-shot wall timing can't see the ~100us kernel).
    """
    S = tok // P            # token tiles per core
    n_groups = S // GT
    n_batches = n_groups // AG
    NT = AG * GT            # tiles per assembly batch (8)

    nc = bass.Bass("TRN2", num_devices=N_CORES)
    embs_d = nc.dram_tensor("embs", [tok, E], F32, kind="ExternalInput")
    w_d = nc.dram_tensor("wt", [P, ECH * NCLS], F32, kind="ExternalInput")
    b_d = nc.dram_tensor("bias", [NCLS, 1], F32, kind="ExternalInput")
    id_d = nc.dram_tensor("ident", [P, P], F32, kind="ExternalInput")
    out_d = nc.dram_tensor("out", [tok, OUTC], F32, kind="ExternalOutput")

    with tile.TileContext(nc) as tc, ExitStack() as ctx:
        consts = ctx.enter_context(tc.tile_pool(name="consts", bufs=1))
        emb_pool = ctx.enter_context(tc.tile_pool(name="emb", bufs=8))
        embT_pool = ctx.enter_context(tc.tile_pool(name="embT", bufs=2))
        expT_pool = ctx.enter_context(tc.tile_pool(name="expT", bufs=2))
        small = ctx.enter_context(tc.tile_pool(name="small", bufs=2))
        outsb = ctx.enter_context(tc.tile_pool(name="outsb", bufs=2))
        psT_pool = ctx.enter_context(tc.tile_pool(name="psT", bufs=4, space="PSUM"))
        psmm_pool = ctx.enter_context(tc.tile_pool(name="psmm", bufs=2, space="PSUM"))
        psxb_pool = ctx.enter_context(tc.tile_pool(name="psxb", bufs=2, space="PSUM"))

        w_raw = consts.tile([P, ECH * NCLS], F32)
        nc.sync.dma_start(w_raw[:], w_d.ap())
        b_sb = consts.tile([NCLS, 1], F32)
        nc.sync.dma_start(b_sb[:], b_d.ap())
        id_sb = consts.tile([P, P], F32)
        nc.sync.dma_start(id_sb[:], id_d.ap())
        w_sb = consts.tile([P, ECH * NCLS], mm_dtype)
        if mm_dtype is F32:
            w_sb = w_raw
        else:
            nc.vector.tensor_copy(w_sb[:], w_raw[:])

        embs_v = embs_d.ap().rearrange("(p i) e -> p i e", p=P, i=S)
        out_v = out_d.ap().rearrange("(p i) c -> p i c", p=P, i=S)

        loop_ctx = tc.For_i(0, loop_reps, 1) if loop_reps else None
        if loop_ctx is not None:
            ctx.enter_context(loop_ctx)

        for ab in range(n_batches * passes):
            ab = ab % n_batches
            ps_xb = psxb_pool.tile([P, NT * NCLS], F32)
            for g2 in range(AG):
                g = ab * AG + g2
                embT = embT_pool.tile([P, ECH * GTOK], mm_dtype)
                embT_v = embT[:].rearrange("p (j t) -> p j t", t=GTOK)
                copy_k = 0
                for c in range(GT):
                    i = g * GT + c
                    emb_nat = emb_pool.tile([P, E], F32)
                    nc.sync.dma_start(emb_nat[:], embs_v[:, i, :])
                    for half in range(2):
                        psT = psT_pool.tile([P, 4 * P], F32)
                        for jj in range(4):
                            j = half * 4 + jj
                            src = emb_nat[:, j * P:(j + 1) * P]
                            if tr_dtype is not F32:
                                src = src.bitcast(tr_dtype)
                            nc.tensor.matmul(
                                psT[:, jj * P:(jj + 1) * P].bitcast(tr_dtype)
                                if tr_dtype is not F32
                                else psT[:, jj * P:(jj + 1) * P],
                                src,
                                id_sb[:].bitcast(tr_dtype)
                                if tr_dtype is not F32
                                else id_sb[:],
                                is_transpose=True,
                            )
                        dst = embT_v[:, half * 4:(half + 1) * 4, c * P:(c + 1) * P]
                        src = psT[:].rearrange("p (j t) -> p j t", j=4)
                        if copy_k < copy_split:
                            nc.scalar.copy(dst, src)
                        else:
                            nc.vector.tensor_copy(dst, src)
                        copy_k += 1
                ps_mm = psmm_pool.tile([NCLS, GTOK], F32)
                for j in range(ECH):
                    nc.tensor.matmul(
                        ps_mm[:],
                        w_sb[:, j * NCLS:(j + 1) * NCLS],
                        embT[:, j * GTOK:(j + 1) * GTOK],
                        start=(j == 0),
                        stop=(j == ECH - 1),
                    )
                expT = expT_pool.tile([NCLS, GTOK], F32)
                nc.scalar.activation(expT[:], ps_mm[:], EXP, bias=b_sb[:], scale=1.0)
                for c in range(GT):
                    it = g2 * GT + c
                    nc.tensor.matmul(
                        ps_xb[:, it * NCLS:(it + 1) * NCLS],
                        expT[:, c * P:(c + 1) * P],
                        id_sb[0:NCLS, 0:NCLS],
                        is_transpose=True,
                    )

            # ---- assembly for NT tiles (tokens p*64 + ab*NT .. +NT) ----
            X = ps_xb[:].rearrange("p (i c) -> p i c", c=NCLS)  # [128, NT, 35]
            sums = small.tile([P, 2 * NT], F32)
            nc.vector.reduce_sum(sums[:, 0:NT], X[:, :, 0:NS], axis=mybir.AxisListType.X)
            nc.vector.reduce_sum(
                sums[:, NT:2 * NT], X[:, :, NS:NCLS], axis=mybir.AxisListType.X
            )
            inv = small.tile([P, 2 * NT], F32)
            nc.vector.reciprocal(inv[:], sums[:])
            inv_s = inv[:, 0:NT]
            inv_f = inv[:, NT:2 * NT]
            dd = small.tile([P, NT], F32)
            nc.vector.tensor_mul(dd[:], inv_s, inv_f)
            cc = small.tile([P, 2 * NT], F32)
            nc.vector.tensor_mul(cc[:, 0:NT], X[:, :, 4], dd[:])  # book scale
            nc.vector.tensor_mul(cc[:, NT:2 * NT], X[:, :, 3], dd[:])  # change
            o_sb = outsb.tile([P, NT * OUTC], F32)
            O = o_sb[:].rearrange("p (i c) -> p i c", c=OUTC)
            inv_s3 = inv_s.unsqueeze(2)
            nc.vector.tensor_mul(O[:, :, 0:1], X[:, :, 0:1], inv_s3)
            nc.vector.tensor_mul(O[:, :, 1:2], X[:, :, 2:3], inv_s3)
            nc.vector.tensor_mul(O[:, :, 2:3], X[:, :, 1:2], inv_s3)
            nc.vector.tensor_mul(
                O[:, :, 3:3 + NF],
                X[:, :, NS:NCLS],
                cc[:, 0:NT].unsqueeze(2).broadcast_to((P, NT, NF)),
            )
            nc.vector.tensor_mul(
                O[:, :, 3 + NF:OUTC],
                X[:, :, NS:NCLS],
                cc[:, NT:2 * NT].unsqueeze(2).broadcast_to((P, NT, NF)),
            )
            nc.gpsimd.dma_start(out_v[:, ab * NT:(ab + 1) * NT, :], o_sb[:].rearrange("p (i c) -> p i c", c=OUTC))

    _split_multiwait(nc)
    return nc


def host_inputs(W_status, b_status, W_flight, b_flight):
    W = np.concatenate([np.asarray(W_status), np.asarray(W_flight)], axis=0)
    W = np.ascontiguousarray(W, dtype=np.float32)          # [35, 1024]
    # w_host[p, j*35 + c] = W[c, j*128 + p]
    w_host = np.ascontiguousarray(
        W.T.reshape(ECH, P, NCLS).transpose(1, 0, 2).reshape(P, ECH * NCLS)
    )
    b_host = np.ascontiguousarray(
        np.concatenate([np.asarray(b_status), np.asarray(b_flight)]).reshape(NCLS, 1),
        dtype=np.float32,
    )
    ident = np.eye(P, dtype=np.float32)
    return w_host, b_host, ident


def kernel(embs, W_status, b_status, W_flight, b_flight, **run_kwargs):
    embs = np.ascontiguousarray(np.asarray(embs), dtype=np.float32)
    tok = embs.shape[0] * embs.shape[1] // N_CORES
    w_host, b_host, ident = host_inputs(W_status, b_status, W_flight, b_flight)

    nc = build_program(tok)

    embs_flat = embs.reshape(-1, E)
    in_maps = [
        {
            "embs": embs_flat[c * tok:(c + 1) * tok],
            "wt": w_host,
            "bias": b_host,
            "ident": ident,
        }
        for c in range(N_CORES)
    ]
    res = run_bass_kernel_spmd(
        nc, in_maps, core_ids=list(range(N_CORES)), **run_kwargs
    )
    out = np.concatenate([res.results[c]["out"] for c in range(N_CORES)], axis=0)
    out = out.reshape(embs.shape[0], embs.shape[1], OUTC)
    if run_kwargs:
        return out, res
    return out
